# revision 8
# baseline (speedup 1.0000x reference)
"""Trainium2 Bass kernel for MemorizingGPT (retrieval_knn).

Sharding: head-parallel across 8 cores. Core c handles batch b=c//4 and the 4
heads hg=c%4 (global heads 4*hg..4*hg+3). Each core computes q/k/v projections
for its head slice over the full sequence, full causal attention for its heads,
the KNN memory attention for its head slice (db is shipped column-sliced per
core), the gated combine, and a partial output projection (contracting only its
256 channels). The host sums the 4 partial projections per batch and adds the
bias terms (bproj and the foldable v-bias contribution).

All matmul inputs are bf16 (fp32 matmul is 4x slower on the PE); PSUM
accumulation stays fp32. Scores are computed transposed [key, q] so that:
  - softmax denominators come free from a ones-column appended to V
  - the attention output lands directly in the [channel, token] layout the
    output projection needs as its stationary operand (no transposes of att).
exp() is applied without a running-max pass (scores here are O(1), far from
fp32 exp overflow).
"""

import numpy as np
import ml_dtypes

import concourse.bass as bass
import concourse.bacc as bacc
import concourse.mybir as mybir
import concourse.tile as tile
from concourse.bass import IndirectOffsetOnAxis
from concourse.masks import make_identity

BF16 = mybir.dt.bfloat16
F32 = mybir.dt.float32
F32R = mybir.dt.float32r
I32 = mybir.dt.int32
AF = mybir.ActivationFunctionType
ALU = mybir.AluOpType

# Problem shapes (hardcoded per the harness contract).
B, T, C = 2, 2048, 1024
N_HEAD = 16
D = 64                      # head dim
K = 3                       # knn neighbors
N_MEM = 131072
N_CORES = 8
HPC = 4                     # heads per core
HS = HPC * D                # per-core head slice of C (256)
DBROW = 2 * HS              # sliced db row: k(256) + v(256) elems


def _ap(base, dims, pdim=None):
    """Custom free-dim access pattern on top of a sliced AP.

    base: AP whose offset marks the starting element (its partition dim is
    kept unless pdim overrides it); dims: [step, count] pairs for free dims.
    """
    p = list(base.ap[0]) if pdim is None else list(pdim)
    return bass.AP(tensor=base.tensor, offset=base.offset,
                   ap=[p] + [[s, n] for s, n in dims])


def build_program(t=T, n_mem=N_MEM, dbg=False):
    """Build the SPMD Bass program (identical on all 8 cores)."""
    nc = bacc.Bacc()
    tg = t // 128            # token groups / key tiles
    nqh = max(1, t // 1024)  # 1024-wide query spans
    qspan = min(t, 1024)
    kt_per_qh = qspan // 128
    nch = max(1, t // 512)   # 512-wide chunks of t

    # ---- dram params (per-core inputs) ----
    xT_d = nc.declare_dram_parameter("xT", [C, t], BF16, isOutput=False)
    wq_d = nc.declare_dram_parameter("wq", [C, HS], BF16, isOutput=False)
    wk_d = nc.declare_dram_parameter("wk", [C, HS], BF16, isOutput=False)
    wv_d = nc.declare_dram_parameter("wv", [C, HS], BF16, isOutput=False)
    wp_d = nc.declare_dram_parameter("wp", [HS, C], BF16, isOutput=False)
    qkb_d = nc.declare_dram_parameter("qkb", [128, 4], F32, isOutput=False)
    dbs_d = nc.declare_dram_parameter("dbs", [n_mem, DBROW], BF16, isOutput=False)
    idx_d = nc.declare_dram_parameter("idx", [128, tg * K], I32, isOutput=False)
    # gate vectors: gpp[:,f] = gate for channel rows of feat-tile f (f=0,1);
    # g1pp[:,h] = (1-gate_h) replicated down 128 partitions.
    gpp_d = nc.declare_dram_parameter("gpp", [128, 2], F32, isOutput=False)
    g1pp_d = nc.declare_dram_parameter("g1pp", [128, HPC], F32, isOutput=False)
    out_d = nc.declare_dram_parameter("out", [t, C], BF16, isOutput=True)
    dbg_d = {}
    if dbg:
        for nm, shape in [("d_qT", [128, 2 * t]), ("d_kT", [128, 2 * t]),
                          ("d_vaug", [128, (t // 128) * HPC * (D + 1)]),
                          ("d_qtok", [128, (t // 128) * HS]),
                          ("d_mem0", [128, K * DBROW]),
                          ("d_mqkv", [128, (t // 128) * HS]),
                          ("d_ypair", [128, 2 * t]),
                          ("d_memT", [128, 2 * t]),
                          ("d_comb", [128, 2 * t]),
                          ("d_yr0", [D, min(t, 1024)]),
                          ("d_rec0", [1, min(t, 1024)]),
                          ("d_bc0", [D, min(t, 1024)]),
                          ("d_pso0", [128, min(t, 1024)])]:
            dbg_d[nm] = nc.declare_dram_parameter(nm, shape, F32, isOutput=True)

    with tile.TileContext(nc) as tc:
        with (
            tc.tile_pool(name="singles", bufs=1) as singles,
            tc.tile_pool(name="bigs", bufs=1) as bigs,
            tc.tile_pool(name="memp", bufs=1) as memp,
            tc.tile_pool(name="tmpp", bufs=2) as tmpp,
        ):
            # ---- resident SBUF tensors ----
            wq_s = singles.tile([128, 8, HS], BF16)
            wk_s = singles.tile([128, 8, HS], BF16)
            wv_s = singles.tile([128, 8, HS], BF16)
            wp_s = singles.tile([128, 2, C], BF16)
            qkb_s = singles.tile([128, 4], F32)
            idx_s = singles.tile([128, tg * K], I32)
            gpp_s = singles.tile([128, 2], F32)
            g1pp_s = singles.tile([128, HPC], F32)

            ident_s = singles.tile([128, 128], BF16)
            ident32_s = singles.tile([128, 128], F32)
            qT_s = bigs.tile([128, 2, t], BF16)
            kT_s = bigs.tile([128, 2, t], BF16)
            vaug_s = bigs.tile([128, tg, HPC * (D + 1)], BF16)
            qtok_s = bigs.tile([128, tg, HS], BF16)
            mqkv_s = bigs.tile([128, tg, HS], F32)
            ypair_s = bigs.tile([128, 2, t], BF16)
            memT_s = bigs.tile([128, 2, t], BF16)
            comb_s = bigs.tile([128, 2, t], BF16)
            qkall_s = bigs.tile([128, tg, K * HPC], F32)
            attall_s = bigs.tile([128, tg, K * HPC], BF16)
            msums_s = bigs.tile([128, tg, HPC], F32)
            mrec_s = bigs.tile([128, tg, HPC], F32)


            make_identity(nc, ident_s[:])
            make_identity(nc, ident32_s[:])
            nc.vector.memset(vaug_s[:], 1.0)            # ============ phase A: qkv + gathers + knn scores ============
            with (
                tc.tile_pool(name="psA", bufs=3, space="PSUM") as psA,
                tc.tile_pool(name="psV", bufs=2, space="PSUM") as psV,
                tc.tile_pool(name="psT", bufs=2, space="PSUM") as psT,
                tc.tile_pool(name="xtp", bufs=1) as xtp,
            ):
                xT_s = xtp.tile([128, 8, t], BF16)
                for i in range(8):
                    nc.sync.dma_start(out=xT_s[:, i, :],
                                      in_=xT_d[i * 128:(i + 1) * 128, :])
                for i in range(8):
                    nc.sync.dma_start(out=wq_s[:, i, :], in_=wq_d[i * 128:(i + 1) * 128, :])
                    nc.sync.dma_start(out=wk_s[:, i, :], in_=wk_d[i * 128:(i + 1) * 128, :])
                    nc.sync.dma_start(out=wv_s[:, i, :], in_=wv_d[i * 128:(i + 1) * 128, :])
                for i in range(2):
                    nc.sync.dma_start(out=wp_s[:, i, :], in_=wp_d[i * 128:(i + 1) * 128, :])
                nc.sync.dma_start(out=qkb_s[:], in_=qkb_d[:])
                nc.sync.dma_start(out=gpp_s[:], in_=gpp_d[:])
                nc.sync.dma_start(out=g1pp_s[:], in_=g1pp_d[:])
                # idx last: delays gather start until weights/x are resident
                nc.sync.dma_start(out=idx_s[:], in_=idx_d[:])
                # all knn gathers issued up front (8-deep rotation per k);
                # HW indirect-DMA contract: one index per partition, dest =
                # whole contiguous tile.
                mems_all = []
                for g in range(tg):
                    mems = []
                    for kk in range(K):
                        mem = memp.tile([128, DBROW], BF16,
                                        tag=f"mem{kk}_{g}", bufs=1,
                                        name=f"mem_{g}_{kk}")
                        nc.gpsimd.indirect_dma_start(
                            out=mem[:],
                            out_offset=None,
                            in_=dbs_d[:],
                            in_offset=IndirectOffsetOnAxis(
                                ap=idx_s[:, g * K + kk:g * K + kk + 1], axis=0),
                        )
                        mems.append(mem)
                    mems_all.append(mems)
                if dbg:
                    for kk in range(K):
                        nc.gpsimd.dma_start(
                            out=dbg_d["d_mem0"][:, kk * DBROW:(kk + 1) * DBROW],
                            in_=mems_all[0][kk][:])

                # q^T then k^T: [feat, tok] = W[:, cols].T @ x^T, bias fused
                for w_s, dst, bcol in ((wq_s, qT_s, 0), (wk_s, kT_s, 2)):
                    for f in range(2):
                        for ch in range(nch):
                            n = min(512, t)
                            ps = psA.tile([128, 512], F32, tag='a')
                            for p in range(8):
                                nc.tensor.matmul(
                                    ps[:, 0:n],
                                    lhsT=w_s[:, p, f * 128:(f + 1) * 128],
                                    rhs=xT_s[:, p, ch * 512:ch * 512 + n],
                                    start=(p == 0), stop=(p == 7),
                                )
                            nc.scalar.add(
                                out=dst[:, f, ch * 512:ch * 512 + n],
                                in_=ps[:, 0:n],
                                add=qkb_s[:, bcol + f:bcol + f + 1],
                            )
                    if dst is qT_s:
                        # q in token layout right away (feeds the knn path)
                        for g in range(tg):
                            for f in range(2):
                                ps = psT.tile([128, 128], BF16, tag="tp")
                                nc.tensor.transpose(
                                    out=ps[:],
                                    in_=qT_s[:, f, g * 128:(g + 1) * 128],
                                    identity=ident_s[:],
                                )
                                nc.vector.tensor_copy(
                                    out=qtok_s[:, g, f * 128:(f + 1) * 128],
                                    in_=ps[:])
                # v: [tok, feat]; no bias (folded into host-side output bias)
                for g in range(tg):
                    ps = psV.tile([128, HS], F32)
                    for p in range(8):
                        nc.tensor.matmul(
                            ps[:],
                            lhsT=xT_s[:, p, g * 128:(g + 1) * 128],
                            rhs=wv_s[:, p, :],
                            start=(p == 0), stop=(p == 7),
                        )
                    nc.vector.tensor_copy(
                        out=_ap(vaug_s[:, g, 0:1], [[D + 1, HPC], [1, D]]),
                        in_=ps[:, 0:HS],
                    )

                # knn pass 1: qk scores per group, then ONE batched
                # exp / k-sum / reciprocal / normalize for all groups.
                for g in range(tg):
                    mems = mems_all[g]
                    tmp1 = tmpp.tile([128, K, HS], BF16, name="tmp1")
                    for kk in range(K):
                        nc.vector.tensor_tensor(
                            out=tmp1[:, kk, :], in0=qtok_s[:, g, 0:HS],
                            in1=mems[kk][:, 0:HS], op=ALU.mult)
                    nc.vector.tensor_reduce(
                        out=qkall_s[:, g, :],
                        in_=tmp1[:].rearrange("p k (h d) -> p (k h) d", d=D),
                        axis=mybir.AxisListType.X, op=ALU.add)
                nc.scalar.activation(
                    out=attall_s[:].rearrange("p g x -> p (g x)"),
                    in_=qkall_s[:].rearrange("p g x -> p (g x)"),
                    func=AF.Exp, scale=0.125)
                att_ghk = _ap(attall_s[:, 0, 0:1],
                              [[K * HPC, tg], [1, HPC], [HPC, K]])
                nc.vector.tensor_reduce(
                    out=msums_s[:].rearrange("p g h -> p (g h)"), in_=att_ghk,
                    axis=mybir.AxisListType.X, op=ALU.add)
                nc.vector.reciprocal_approx_fast(
                    out=mrec_s[:].rearrange("p g h -> p (g h)"),
                    in_=msums_s[:].rearrange("p g h -> p (g h)"))
                rec_rep = _ap(mrec_s[:, 0, 0:1],
                              [[HPC, tg], [0, K], [1, HPC]])
                nc.vector.tensor_tensor(
                    out=attall_s[:].rearrange("p g x -> p (g x)"),
                    in0=attall_s[:].rearrange("p g x -> p (g x)"),
                    in1=rec_rep, op=ALU.mult)

                # knn pass 2: weighted value sum per group
                for g in range(tg):
                    mems = mems_all[g]
                    tmp2 = tmpp.tile([128, HS, K], BF16, name="tmp2")
                    for kk in range(K):
                        attn_rep = _ap(attall_s[:, g, kk * HPC:kk * HPC + 1],
                                       [[1, HPC], [0, D]])
                        eng = nc.gpsimd if kk < 2 else nc.vector
                        eng.tensor_tensor(
                            out=tmp2[:, :, kk].rearrange("p (h d) -> p h d", d=D),
                            in0=attn_rep,
                            in1=mems[kk][:, HS:2 * HS].rearrange(
                                "p (h d) -> p h d", d=D),
                            op=ALU.mult)
                    nc.vector.tensor_reduce(
                        out=mqkv_s[:, g, :], in_=tmp2[:],
                        axis=mybir.AxisListType.X, op=ALU.add)

            # ================= phase B: causal attention =================
            with (
                tc.tile_pool(name="psS", bufs=2, space="PSUM") as psS,
                tc.tile_pool(name="psO", bufs=2, space="PSUM") as psO,
                tc.tile_pool(name="expp", bufs=4) as expp,
                tc.tile_pool(name="yrawp", bufs=2) as yrawp,
                tc.tile_pool(name="recp", bufs=2) as recp,
                tc.tile_pool(name="bcp", bufs=2) as bcp,
                tc.tile_pool(name="outp", bufs=2) as outp,
            ):
                for hp in range(2):
                    for qh in range(nqh):
                        psos = {}
                        for h in (2 * hp, 2 * hp + 1):
                            psos[h] = psO.tile([128, qspan], F32, tag='o',
                                               name=f"pso_{h}_{qh}")
                        nkt = kt_per_qh * (qh + 1)
                        for kt in range(nkt):
                            qlo = max(qspan * qh, 128 * kt)
                            qhi = qspan * (qh + 1)
                            nq = qhi - qlo
                            # score matmuls for the head pair emitted
                            # back-to-back: base partitions 0/64 map to PE
                            # row-groups 0/64 (tile_position auto-derived),
                            # so the two K=64 matmuls run concurrently.
                            psss, expts = {}, {}
                            for h in (2 * hp, 2 * hp + 1):
                                f, r0 = h // 2, (h % 2) * D
                                pss = psS.tile([128, qspan], F32, tag='s',
                                               name="pss")
                                psss[h] = pss
                                for j in range((nq + 511) // 512):
                                    nj = min(512, nq - j * 512)
                                    nc.tensor.matmul(
                                        pss[:, j * 512:j * 512 + nj],
                                        lhsT=kT_s[r0:r0 + D, f, kt * 128:(kt + 1) * 128],
                                        rhs=qT_s[r0:r0 + D, f, qlo + j * 512:qlo + j * 512 + nj],
                                        start=True, stop=True)
                            for h in (2 * hp, 2 * hp + 1):
                                expt = expp.tile([128, qspan], BF16, name="expt")
                                expts[h] = expt
                                nc.scalar.activation(out=expt[:, 0:nq],
                                                     in_=psss[h][:, 0:nq],
                                                     func=AF.Exp, scale=0.125)
                                if kt >= kt_per_qh * qh:
                                    nc.gpsimd.affine_select(
                                        out=expt[:, 0:128], in_=expt[:, 0:128],
                                        compare_op=ALU.is_ge, fill=0.0,
                                        base=0, pattern=[[1, 128]],
                                        channel_multiplier=-1)
                            for h in (2 * hp, 2 * hp + 1):
                                pso = psos[h]
                                expt = expts[h]
                                cs = min(512, qspan)
                                for ab in range(qspan // cs):
                                    lo = max(qlo, qspan * qh + cs * ab)
                                    n = qspan * qh + cs * (ab + 1) - lo
                                    if n <= 0:
                                        continue
                                    last_kt = min(
                                        nkt - 1,
                                        (qspan * qh + cs * (ab + 1)) // 128 - 1)
                                    nc.tensor.matmul(
                                        pso[0:D + 1, lo - qspan * qh:lo - qspan * qh + n],
                                        lhsT=vaug_s[:, kt, h * (D + 1):(h + 1) * (D + 1)],
                                        rhs=expt[:, lo - qlo:lo - qlo + n],
                                        start=(kt == 0), stop=(kt == last_kt))
                        # epilogue: y_norm * (1-gate) into ypair
                        for h in (2 * hp, 2 * hp + 1):
                            f, r0 = h // 2, (h % 2) * D
                            pso = psos[h]
                            # on vector, not scalar: the ACT engine is the
                            # critical chain during attention (exp stream)
                            yr = yrawp.tile([D, qspan], BF16, name="yr")
                            nc.vector.tensor_scalar_mul(
                                out=yr[:], in0=pso[0:D, :],
                                scalar1=g1pp_s[0:D, h:h + 1])
                            # custom-DVE ops misread PSUM: stage sums in SBUF
                            sums_sb = recp.tile([1, qspan], F32, tag="r",
                                                name="sums_sb")
                            nc.vector.tensor_copy(out=sums_sb[:],
                                                  in_=pso[D:D + 1, :])
                            rec = recp.tile([1, qspan], F32, tag="r", name="rec")
                            nc.vector.reciprocal_approx_fast(
                                out=rec[:], in_=sums_sb[:])
                            bc = bcp.tile([D, qspan], F32, name="bc")
                            nc.gpsimd.partition_broadcast(bc[:], rec[:],
                                                          channels=D)
                            nc.vector.tensor_tensor(
                                out=ypair_s[r0:r0 + D, f, qh * qspan:(qh + 1) * qspan],
                                in0=yr[:], in1=bc[:], op=ALU.mult)
                            if dbg and h == 0 and qh == 0:
                                nc.gpsimd.dma_start(out=dbg_d["d_yr0"][:], in_=yr[:])
                                nc.gpsimd.dma_start(out=dbg_d["d_rec0"][:], in_=rec[:])
                                nc.gpsimd.dma_start(out=dbg_d["d_bc0"][:], in_=bc[:])
                                pcp = bcp.tile([128, qspan], F32, name="pcp")
                                nc.vector.tensor_copy(out=pcp[:], in_=pso[:])
                                nc.gpsimd.dma_start(out=dbg_d["d_pso0"][:], in_=pcp[:])

                # mem^T (transposes use attention-psum slots), scale by gate
                for g in range(tg):
                    for f in range(2):
                        ps = psS.tile([128, qspan], F32, tag='s', name="pst2")
                        nc.tensor.transpose(
                            out=ps[:, 0:128],
                            in_=mqkv_s[:, g, f * 128:(f + 1) * 128],
                            identity=ident32_s[:])
                        nc.vector.tensor_scalar_mul(
                            out=memT_s[:, f, g * 128:(g + 1) * 128],
                            in0=ps[:, 0:128], scalar1=gpp_s[:, f:f + 1])

                # ============== combine + output projection ==============
                for f in range(2):
                    for qh in range(nqh):
                        sl = slice(qh * qspan, (qh + 1) * qspan)
                        nc.vector.tensor_tensor(
                            out=comb_s[:, f, sl], in0=ypair_s[:, f, sl],
                            in1=memT_s[:, f, sl], op=ALU.add)
                if dbg:
                    for nm, src in [("d_qT", qT_s), ("d_kT", kT_s),
                                    ("d_vaug", vaug_s), ("d_qtok", qtok_s),
                                    ("d_mqkv", mqkv_s), ("d_ypair", ypair_s),
                                    ("d_memT", memT_s), ("d_comb", comb_s)]:
                        nc.gpsimd.dma_start(out=dbg_d[nm][:], in_=src[:])
                for g in range(tg):
                    psp = psS.tile([128, qspan], F32, tag='s', name="psp")
                    nco = min(qspan, C)
                    csp = min(512, nco)
                    for co in range(0, C, nco):
                        for ab in range(nco // csp):
                            for p2 in range(2):
                                nc.tensor.matmul(
                                    psp[:, ab * csp:(ab + 1) * csp],
                                    lhsT=comb_s[:, p2, g * 128:(g + 1) * 128],
                                    rhs=wp_s[:, p2, co + ab * csp:co + (ab + 1) * csp],
                                    start=(p2 == 0), stop=(p2 == 1))
                        ot = outp.tile([128, qspan], BF16, name="ot")
                        if g % 2 == 0:
                            nc.scalar.copy(out=ot[:, 0:nco], in_=psp[:, 0:nco])
                        else:
                            nc.vector.tensor_copy(out=ot[:, 0:nco],
                                                  in_=psp[:, 0:nco])
                        nc.sync.dma_start(
                            out=out_d[g * 128:(g + 1) * 128, co:co + nco],
                            in_=ot[:, 0:nco])
    nc.finalize()
    return nc


def host_prepare(inputs, t=T, n_mem=N_MEM):
    """Build the 8 per-core input maps + the host-side output bias."""
    bf = ml_dtypes.bfloat16
    x = np.asarray(inputs["x"], np.float32)
    Wqkv = np.asarray(inputs["Wqkv"], np.float32)
    bqkv = np.asarray(inputs["bqkv"], np.float32)
    Wproj = np.asarray(inputs["Wproj"], np.float32)
    bproj = np.asarray(inputs["bproj"], np.float32)
    gate = np.asarray(inputs["gate_bias"], np.float32).reshape(N_HEAD)
    db = np.asarray(inputs["db"], np.float32)
    indices = np.asarray(inputs["indices"])

    tg = t // 128
    in_maps = []
    for c in range(N_CORES):
        b, hg = c // 4, c % 4
        cols = slice(hg * HS, (hg + 1) * HS)
        xT = np.ascontiguousarray(x[b].T).astype(bf)                   # [C, t]
        wq = Wqkv[:, cols].astype(bf)
        wk = Wqkv[:, C:][:, cols].astype(bf)
        wv = Wqkv[:, 2 * C:][:, cols].astype(bf)
        wp = Wproj[cols, :].astype(bf)                                  # [HS, C]
        bq = bqkv[cols].astype(np.float32)
        bk = bqkv[C:][cols].astype(np.float32)
        qkb = np.stack([bq[:128], bq[128:], bk[:128], bk[128:]], axis=1)
        dbs = np.ascontiguousarray(db[:, :, cols]).reshape(n_mem, DBROW).astype(bf)
        idx = np.ascontiguousarray(
            indices[b].reshape(tg, 128, K).transpose(1, 0, 2).reshape(128, tg * K)
        ).astype(np.int32)
        ghead = gate[hg * HPC:(hg + 1) * HPC]                           # [4]
        gpp = np.stack([np.repeat(ghead[0:2], D), np.repeat(ghead[2:4], D)],
                       axis=1).astype(np.float32)                       # [128,2]
        g1pp = np.tile((1.0 - ghead)[None, :], (128, 1)).astype(np.float32)
        in_maps.append(dict(xT=xT, wq=wq, wk=wk, wv=wv, wp=wp, qkb=qkb,
                            dbs=dbs, idx=idx, gpp=gpp, g1pp=g1pp))

    # host-side bias: bproj + ((1-gate) * bv) @ Wproj
    gexp = np.repeat(gate, D)                                           # [C]
    bv = bqkv[2 * C:]
    host_bias = bproj + ((1.0 - gexp) * bv) @ Wproj                     # [C]
    return in_maps, host_bias


def host_finalize(results, host_bias, t=T):
    out = np.zeros((B, t, C), np.float32)
    for b in range(B):
        acc = np.zeros((t, C), np.float64)
        for hg in range(4):
            acc += results[b * 4 + hg]["out"].astype(np.float64)
        out[b] = (acc + host_bias[None, :]).astype(np.float32)
    return out


_CACHED_NC = None


def kernel(**inputs) -> np.ndarray:
    global _CACHED_NC
    from concourse.bass_utils import run_bass_kernel_spmd
    if _CACHED_NC is None:
        _CACHED_NC = build_program()
    in_maps, host_bias = host_prepare(inputs)
    res = run_bass_kernel_spmd(_CACHED_NC, in_maps, list(range(N_CORES)))
    return host_finalize(res.results, host_bias)



# revision 12
# speedup vs baseline: 1.3833x; 1.3833x over previous
"""Trainium2 Bass kernel for MemorizingGPT (retrieval_knn).

Sharding: head-parallel across 8 cores. Core c handles batch b=c//4 and the 4
heads hg=c%4 (global heads 4*hg..4*hg+3). Each core computes q/k/v projections
for its head slice over the full sequence, full causal attention for its heads,
the KNN memory attention for its head slice (db is shipped column-sliced per
core), the gated combine, and a partial output projection (contracting only its
256 channels). The host sums the 4 partial projections per batch and adds the
bias terms (bproj and the foldable v-bias contribution).

All matmul inputs are bf16 (fp32 matmul is 4x slower on the PE); PSUM
accumulation stays fp32. Scores are computed transposed [key, q] so that:
  - softmax denominators come free from a ones-column appended to V
  - the attention output lands directly in the [channel, token] layout the
    output projection needs as its stationary operand (no transposes of att).
exp() is applied without a running-max pass (scores here are O(1), far from
fp32 exp overflow).
"""

import numpy as np
import ml_dtypes

import concourse.bass as bass
import concourse.bacc as bacc
import concourse.mybir as mybir
import concourse.tile as tile
from concourse.bass import IndirectOffsetOnAxis
from concourse.masks import make_identity

BF16 = mybir.dt.bfloat16
F32 = mybir.dt.float32
F32R = mybir.dt.float32r
I32 = mybir.dt.int32
AF = mybir.ActivationFunctionType
ALU = mybir.AluOpType

# Problem shapes (hardcoded per the harness contract).
B, T, C = 2, 2048, 1024
N_HEAD = 16
D = 64                      # head dim
K = 3                       # knn neighbors
N_MEM = 131072
N_CORES = 8
HPC = 4                     # heads per core
HS = HPC * D                # per-core head slice of C (256)
DBROW = 2 * HS              # sliced db row: k(256) + v(256) elems


def _ap(base, dims, pdim=None):
    """Custom free-dim access pattern on top of a sliced AP.

    base: AP whose offset marks the starting element (its partition dim is
    kept unless pdim overrides it); dims: [step, count] pairs for free dims.
    """
    p = list(base.ap[0]) if pdim is None else list(pdim)
    return bass.AP(tensor=base.tensor, offset=base.offset,
                   ap=[p] + [[s, n] for s, n in dims])


def build_program(t=T, n_mem=N_MEM, dbg=False):
    """Build the SPMD Bass program (identical on all 8 cores)."""
    nc = bacc.Bacc()
    tg = t // 128            # token groups / key tiles
    nqh = max(1, t // 1024)  # 1024-wide query spans
    qspan = min(t, 1024)
    kt_per_qh = qspan // 128
    nch = max(1, t // 512)   # 512-wide chunks of t

    # ---- dram params (per-core inputs) ----
    xT_d = nc.declare_dram_parameter("xT", [C, t], BF16, isOutput=False)
    wq_d = nc.declare_dram_parameter("wq", [C, HS], BF16, isOutput=False)
    wk_d = nc.declare_dram_parameter("wk", [C, HS], BF16, isOutput=False)
    wv_d = nc.declare_dram_parameter("wv", [C, HS], BF16, isOutput=False)
    wp_d = nc.declare_dram_parameter("wp", [HS, C], BF16, isOutput=False)
    qkb_d = nc.declare_dram_parameter("qkb", [128, 4], F32, isOutput=False)
    dbs_d = nc.declare_dram_parameter("dbs", [n_mem, DBROW], BF16, isOutput=False)
    idx_d = nc.declare_dram_parameter("idx", [128, tg * K], I32, isOutput=False)
    # gate vectors: gpp[:,f] = gate for channel rows of feat-tile f (f=0,1);
    # g1pp[:,h] = (1-gate_h) replicated down 128 partitions.
    gpp_d = nc.declare_dram_parameter("gpp", [128, 2], F32, isOutput=False)
    g1pp_d = nc.declare_dram_parameter("g1pp", [128, HPC], F32, isOutput=False)
    out_d = nc.declare_dram_parameter("out", [t, C], BF16, isOutput=True)
    dbg_d = {}
    if dbg:
        for nm, shape in [("d_qT", [128, 2 * t]), ("d_kT", [128, 2 * t]),
                          ("d_vaug", [128, (t // 128) * HPC * (D + 1)]),
                          ("d_qtok", [128, (t // 128) * HS]),
                          ("d_mem0", [128, K * DBROW]),
                          ("d_mqkv", [128, (t // 128) * HS]),
                          ("d_ypair", [128, 2 * t]),
                          ("d_memT", [128, 2 * t]),
                          ("d_comb", [128, 2 * t]),
                          ("d_yr0", [D, min(t, 1024)]),
                          ("d_rec0", [1, min(t, 1024)]),
                          ("d_bc0", [D, min(t, 1024)]),
                          ("d_pso0", [128, min(t, 1024)])]:
            dbg_d[nm] = nc.declare_dram_parameter(nm, shape, F32, isOutput=True)

    with tile.TileContext(nc) as tc:
        with (
            tc.tile_pool(name="singles", bufs=1) as singles,
            tc.tile_pool(name="bigs", bufs=1) as bigs,
            tc.tile_pool(name="memp", bufs=1) as memp,
            tc.tile_pool(name="tmpp", bufs=2) as tmpp,
        ):
            # ---- resident SBUF tensors ----
            wq_s = singles.tile([128, 8, HS], BF16)
            wk_s = singles.tile([128, 8, HS], BF16)
            wv_s = singles.tile([128, 8, HS], BF16)
            wp_s = singles.tile([128, 2, C], BF16)
            qkb_s = singles.tile([128, 4], F32)
            idx_s = singles.tile([128, tg * K], I32)
            gpp_s = singles.tile([128, 2], F32)
            g1pp_s = singles.tile([128, HPC], F32)

            ident_s = singles.tile([128, 128], BF16)
            ident32_s = singles.tile([128, 128], F32)
            qT_s = bigs.tile([128, 2, t], BF16)
            kT_s = bigs.tile([128, 2, t], BF16)
            vaug_s = bigs.tile([128, tg, HPC * (D + 1)], BF16)
            qtok_s = bigs.tile([128, tg, HS], BF16)
            mqkv_s = bigs.tile([128, tg, HS], F32)
            ypair_s = bigs.tile([128, 2, t], BF16)
            memT_s = bigs.tile([128, 2, t], BF16)
            comb_s = bigs.tile([128, 2, t], BF16)
            qkall_s = bigs.tile([128, tg, K * HPC], F32)
            attall_s = bigs.tile([128, tg, K * HPC], BF16)
            msums_s = bigs.tile([128, tg, HPC], F32)
            mrec_s = bigs.tile([128, tg, HPC], F32)


            make_identity(nc, ident_s[:])
            make_identity(nc, ident32_s[:])
            nc.vector.memset(vaug_s[:], 1.0)            # ============ phase A: qkv + gathers + knn scores ============
            with (
                tc.tile_pool(name="psA", bufs=3, space="PSUM") as psA,
                tc.tile_pool(name="psV", bufs=2, space="PSUM") as psV,
                tc.tile_pool(name="psT", bufs=2, space="PSUM") as psT,
                tc.tile_pool(name="xtp", bufs=1) as xtp,
            ):
                xT_s = xtp.tile([128, 8, t], BF16)
                # idx + gathers FIRST: the 48 serialized indirect DMAs
                # (~1.6us each) are the longest dependency chain feeding the
                # knn path; they must start at t=0, concurrent with the x
                # and weight loads which use different DMA queues.
                nc.sync.dma_start(out=idx_s[:], in_=idx_d[:])
                mems_all = []
                for g in range(tg):
                    mems = []
                    for kk in range(K):
                        mem = memp.tile([128, DBROW], BF16,
                                        tag=f"mem{kk}_{g}", bufs=1,
                                        name=f"mem_{g}_{kk}")
                        nc.gpsimd.indirect_dma_start(
                            out=mem[:],
                            out_offset=None,
                            in_=dbs_d[:],
                            in_offset=IndirectOffsetOnAxis(
                                ap=idx_s[:, g * K + kk:g * K + kk + 1], axis=0),
                        )
                        mems.append(mem)
                    mems_all.append(mems)
                for i in range(8):
                    nc.sync.dma_start(out=xT_s[:, i, :],
                                      in_=xT_d[i * 128:(i + 1) * 128, :])
                for i in range(8):
                    nc.sync.dma_start(out=wq_s[:, i, :], in_=wq_d[i * 128:(i + 1) * 128, :])
                    nc.sync.dma_start(out=wk_s[:, i, :], in_=wk_d[i * 128:(i + 1) * 128, :])
                    nc.sync.dma_start(out=wv_s[:, i, :], in_=wv_d[i * 128:(i + 1) * 128, :])
                for i in range(2):
                    nc.sync.dma_start(out=wp_s[:, i, :], in_=wp_d[i * 128:(i + 1) * 128, :])
                nc.sync.dma_start(out=qkb_s[:], in_=qkb_d[:])
                nc.sync.dma_start(out=gpp_s[:], in_=gpp_d[:])
                nc.sync.dma_start(out=g1pp_s[:], in_=g1pp_d[:])
                if dbg:
                    for kk in range(K):
                        nc.gpsimd.dma_start(
                            out=dbg_d["d_mem0"][:, kk * DBROW:(kk + 1) * DBROW],
                            in_=mems_all[0][kk][:])

                # q^T then k^T: [feat, tok] = W[:, cols].T @ x^T, bias fused
                for w_s, dst, bcol in ((wq_s, qT_s, 0), (wk_s, kT_s, 2)):
                    for f in range(2):
                        for ch in range(nch):
                            n = min(512, t)
                            ps = psA.tile([128, 512], F32, tag='a')
                            for p in range(8):
                                nc.tensor.matmul(
                                    ps[:, 0:n],
                                    lhsT=w_s[:, p, f * 128:(f + 1) * 128],
                                    rhs=xT_s[:, p, ch * 512:ch * 512 + n],
                                    start=(p == 0), stop=(p == 7),
                                )
                            nc.scalar.add(
                                out=dst[:, f, ch * 512:ch * 512 + n],
                                in_=ps[:, 0:n],
                                add=qkb_s[:, bcol + f:bcol + f + 1],
                            )
                    if dst is qT_s:
                        # q in token layout right away (feeds the knn path)
                        for g in range(tg):
                            for f in range(2):
                                ps = psT.tile([128, 128], BF16, tag="tp")
                                nc.tensor.transpose(
                                    out=ps[:],
                                    in_=qT_s[:, f, g * 128:(g + 1) * 128],
                                    identity=ident_s[:],
                                )
                                nc.vector.tensor_copy(
                                    out=qtok_s[:, g, f * 128:(f + 1) * 128],
                                    in_=ps[:])
                # v: [tok, feat]; no bias (folded into host-side output bias)
                for g in range(tg):
                    ps = psV.tile([128, HS], F32)
                    for p in range(8):
                        nc.tensor.matmul(
                            ps[:],
                            lhsT=xT_s[:, p, g * 128:(g + 1) * 128],
                            rhs=wv_s[:, p, :],
                            start=(p == 0), stop=(p == 7),
                        )
                    nc.vector.tensor_copy(
                        out=_ap(vaug_s[:, g, 0:1], [[D + 1, HPC], [1, D]]),
                        in_=ps[:, 0:HS],
                    )

                # knn pass 1: qk scores per group, then ONE batched
                # exp / k-sum / reciprocal / normalize for all groups.
                for g in range(tg):
                    mems = mems_all[g]
                    tmp1 = tmpp.tile([128, K, HS], BF16, name="tmp1")
                    for kk in range(K):
                        nc.vector.tensor_tensor(
                            out=tmp1[:, kk, :], in0=qtok_s[:, g, 0:HS],
                            in1=mems[kk][:, 0:HS], op=ALU.mult)
                    nc.vector.tensor_reduce(
                        out=qkall_s[:, g, :],
                        in_=tmp1[:].rearrange("p k (h d) -> p (k h) d", d=D),
                        axis=mybir.AxisListType.X, op=ALU.add)
                nc.scalar.activation(
                    out=attall_s[:].rearrange("p g x -> p (g x)"),
                    in_=qkall_s[:].rearrange("p g x -> p (g x)"),
                    func=AF.Exp, scale=0.125)
                att_ghk = _ap(attall_s[:, 0, 0:1],
                              [[K * HPC, tg], [1, HPC], [HPC, K]])
                nc.vector.tensor_reduce(
                    out=msums_s[:].rearrange("p g h -> p (g h)"), in_=att_ghk,
                    axis=mybir.AxisListType.X, op=ALU.add)
                nc.vector.reciprocal_approx_fast(
                    out=mrec_s[:].rearrange("p g h -> p (g h)"),
                    in_=msums_s[:].rearrange("p g h -> p (g h)"))
                rec_rep = _ap(mrec_s[:, 0, 0:1],
                              [[HPC, tg], [0, K], [1, HPC]])
                nc.vector.tensor_tensor(
                    out=attall_s[:].rearrange("p g x -> p (g x)"),
                    in0=attall_s[:].rearrange("p g x -> p (g x)"),
                    in1=rec_rep, op=ALU.mult)

                # knn pass 2 moved into phase B (emitted interleaved with
                # attention hp=1 so it fills Vector idle time there instead
                # of clogging the FIFO ahead of the hp=0 epilogues).
                def knn_pass2(g0, g1):
                    for g in range(g0, g1):
                        mems = mems_all[g]
                        tmp2 = tmpp.tile([128, HS, K], BF16, name="tmp2")
                        for kk in range(K):
                            attn_rep = _ap(attall_s[:, g, kk * HPC:kk * HPC + 1],
                                           [[1, HPC], [0, D]])
                            nc.vector.tensor_tensor(
                                out=tmp2[:, :, kk].rearrange("p (h d) -> p h d", d=D),
                                in0=attn_rep,
                                in1=mems[kk][:, HS:2 * HS].rearrange(
                                    "p (h d) -> p h d", d=D),
                                op=ALU.mult)
                        nc.vector.tensor_reduce(
                            out=mqkv_s[:, g, :], in_=tmp2[:],
                            axis=mybir.AxisListType.X, op=ALU.add)

            # ================= phase B: causal attention =================
            with (
                tc.tile_pool(name="psS", bufs=2, space="PSUM") as psS,
                tc.tile_pool(name="psO", bufs=2, space="PSUM") as psO,
                tc.tile_pool(name="expp", bufs=4) as expp,
                tc.tile_pool(name="yrawp", bufs=2) as yrawp,
                tc.tile_pool(name="recp", bufs=2) as recp,
                tc.tile_pool(name="bcp", bufs=2) as bcp,
                tc.tile_pool(name="outp", bufs=2) as outp,
            ):
                for hp in range(2):
                    for qh in range(nqh):
                        psos = {}
                        for h in (2 * hp, 2 * hp + 1):
                            psos[h] = psO.tile([128, qspan], F32, tag='o',
                                               name=f"pso_{h}_{qh}")
                        nkt = kt_per_qh * (qh + 1)
                        for kt in range(nkt):
                            qlo = max(qspan * qh, 128 * kt)
                            qhi = qspan * (qh + 1)
                            nq = qhi - qlo
                            # score matmuls for the head pair emitted
                            # back-to-back: base partitions 0/64 map to PE
                            # row-groups 0/64 (tile_position auto-derived),
                            # so the two K=64 matmuls run concurrently.
                            psss, expts = {}, {}
                            for h in (2 * hp, 2 * hp + 1):
                                f, r0 = h // 2, (h % 2) * D
                                pss = psS.tile([128, qspan], F32, tag='s',
                                               name="pss")
                                psss[h] = pss
                                for j in range((nq + 511) // 512):
                                    nj = min(512, nq - j * 512)
                                    nc.tensor.matmul(
                                        pss[:, j * 512:j * 512 + nj],
                                        lhsT=kT_s[r0:r0 + D, f, kt * 128:(kt + 1) * 128],
                                        rhs=qT_s[r0:r0 + D, f, qlo + j * 512:qlo + j * 512 + nj],
                                        start=True, stop=True)
                            for h in (2 * hp, 2 * hp + 1):
                                expt = expp.tile([128, qspan], BF16, name="expt")
                                expts[h] = expt
                                nc.scalar.activation(out=expt[:, 0:nq],
                                                     in_=psss[h][:, 0:nq],
                                                     func=AF.Exp, scale=0.125)
                                if kt >= kt_per_qh * qh:
                                    nc.gpsimd.affine_select(
                                        out=expt[:, 0:128], in_=expt[:, 0:128],
                                        compare_op=ALU.is_ge, fill=0.0,
                                        base=0, pattern=[[1, 128]],
                                        channel_multiplier=-1)
                            for h in (2 * hp, 2 * hp + 1):
                                pso = psos[h]
                                expt = expts[h]
                                cs = min(512, qspan)
                                for ab in range(qspan // cs):
                                    lo = max(qlo, qspan * qh + cs * ab)
                                    n = qspan * qh + cs * (ab + 1) - lo
                                    if n <= 0:
                                        continue
                                    last_kt = min(
                                        nkt - 1,
                                        (qspan * qh + cs * (ab + 1)) // 128 - 1)
                                    nc.tensor.matmul(
                                        pso[0:D + 1, lo - qspan * qh:lo - qspan * qh + n],
                                        lhsT=vaug_s[:, kt, h * (D + 1):(h + 1) * (D + 1)],
                                        rhs=expt[:, lo - qlo:lo - qlo + n],
                                        start=(kt == 0), stop=(kt == last_kt))
                        # epilogue: y_norm * (1-gate) into ypair
                        for h in (2 * hp, 2 * hp + 1):
                            f, r0 = h // 2, (h % 2) * D
                            pso = psos[h]
                            # on vector, not scalar: the ACT engine is the
                            # critical chain during attention (exp stream)
                            yr = yrawp.tile([D, qspan], BF16, name="yr")
                            nc.vector.tensor_scalar_mul(
                                out=yr[:], in0=pso[0:D, :],
                                scalar1=g1pp_s[0:D, h:h + 1])
                            # custom-DVE ops misread PSUM: stage sums in SBUF
                            sums_sb = recp.tile([1, qspan], F32, tag="r",
                                                name="sums_sb")
                            nc.vector.tensor_copy(out=sums_sb[:],
                                                  in_=pso[D:D + 1, :])
                            rec = recp.tile([1, qspan], F32, tag="r", name="rec")
                            nc.vector.reciprocal_approx_fast(
                                out=rec[:], in_=sums_sb[:])
                            bc = bcp.tile([D, qspan], F32, name="bc")
                            nc.gpsimd.partition_broadcast(bc[:], rec[:],
                                                          channels=D)
                            nc.vector.tensor_tensor(
                                out=ypair_s[r0:r0 + D, f, qh * qspan:(qh + 1) * qspan],
                                in0=yr[:], in1=bc[:], op=ALU.mult)
                            if dbg and h == 0 and qh == 0:
                                nc.gpsimd.dma_start(out=dbg_d["d_yr0"][:], in_=yr[:])
                                nc.gpsimd.dma_start(out=dbg_d["d_rec0"][:], in_=rec[:])
                                nc.gpsimd.dma_start(out=dbg_d["d_bc0"][:], in_=bc[:])
                                pcp = bcp.tile([128, qspan], F32, name="pcp")
                                nc.vector.tensor_copy(out=pcp[:], in_=pso[:])
                                nc.gpsimd.dma_start(out=dbg_d["d_pso0"][:], in_=pcp[:])
                        # weave knn pass 2 into hp=1: after each q-block's
                        # epilogue, Vector is idle while the PE grinds the
                        # next block's scores/attv.
                        if hp == 1:
                            knn_pass2(*((0, 12) if qh == 0 else (12, tg)))

                # mem^T (transposes use attention-psum slots), scale by gate
                for g in range(tg):
                    for f in range(2):
                        ps = psS.tile([128, qspan], F32, tag='s', name="pst2")
                        nc.tensor.transpose(
                            out=ps[:, 0:128],
                            in_=mqkv_s[:, g, f * 128:(f + 1) * 128],
                            identity=ident32_s[:])
                        nc.vector.tensor_scalar_mul(
                            out=memT_s[:, f, g * 128:(g + 1) * 128],
                            in0=ps[:, 0:128], scalar1=gpp_s[:, f:f + 1])

                # ============== combine + output projection ==============
                for f in range(2):
                    for qh in range(nqh):
                        sl = slice(qh * qspan, (qh + 1) * qspan)
                        nc.vector.tensor_tensor(
                            out=comb_s[:, f, sl], in0=ypair_s[:, f, sl],
                            in1=memT_s[:, f, sl], op=ALU.add)
                if dbg:
                    for nm, src in [("d_qT", qT_s), ("d_kT", kT_s),
                                    ("d_vaug", vaug_s), ("d_qtok", qtok_s),
                                    ("d_mqkv", mqkv_s), ("d_ypair", ypair_s),
                                    ("d_memT", memT_s), ("d_comb", comb_s)]:
                        nc.gpsimd.dma_start(out=dbg_d[nm][:], in_=src[:])
                for g in range(tg):
                    psp = psS.tile([128, qspan], F32, tag='s', name="psp")
                    nco = min(qspan, C)
                    csp = min(512, nco)
                    for co in range(0, C, nco):
                        for ab in range(nco // csp):
                            for p2 in range(2):
                                nc.tensor.matmul(
                                    psp[:, ab * csp:(ab + 1) * csp],
                                    lhsT=comb_s[:, p2, g * 128:(g + 1) * 128],
                                    rhs=wp_s[:, p2, co + ab * csp:co + (ab + 1) * csp],
                                    start=(p2 == 0), stop=(p2 == 1))
                        ot = outp.tile([128, qspan], BF16, name="ot")
                        if g % 2 == 0:
                            nc.scalar.copy(out=ot[:, 0:nco], in_=psp[:, 0:nco])
                        else:
                            nc.vector.tensor_copy(out=ot[:, 0:nco],
                                                  in_=psp[:, 0:nco])
                        nc.sync.dma_start(
                            out=out_d[g * 128:(g + 1) * 128, co:co + nco],
                            in_=ot[:, 0:nco])
    nc.finalize()
    return nc


def host_prepare(inputs, t=T, n_mem=N_MEM):
    """Build the 8 per-core input maps + the host-side output bias."""
    bf = ml_dtypes.bfloat16
    x = np.asarray(inputs["x"], np.float32)
    Wqkv = np.asarray(inputs["Wqkv"], np.float32)
    bqkv = np.asarray(inputs["bqkv"], np.float32)
    Wproj = np.asarray(inputs["Wproj"], np.float32)
    bproj = np.asarray(inputs["bproj"], np.float32)
    gate = np.asarray(inputs["gate_bias"], np.float32).reshape(N_HEAD)
    db = np.asarray(inputs["db"], np.float32)
    indices = np.asarray(inputs["indices"])

    tg = t // 128
    in_maps = []
    for c in range(N_CORES):
        b, hg = c // 4, c % 4
        cols = slice(hg * HS, (hg + 1) * HS)
        xT = np.ascontiguousarray(x[b].T).astype(bf)                   # [C, t]
        wq = Wqkv[:, cols].astype(bf)
        wk = Wqkv[:, C:][:, cols].astype(bf)
        wv = Wqkv[:, 2 * C:][:, cols].astype(bf)
        wp = Wproj[cols, :].astype(bf)                                  # [HS, C]
        bq = bqkv[cols].astype(np.float32)
        bk = bqkv[C:][cols].astype(np.float32)
        qkb = np.stack([bq[:128], bq[128:], bk[:128], bk[128:]], axis=1)
        dbs = np.ascontiguousarray(db[:, :, cols]).reshape(n_mem, DBROW).astype(bf)
        idx = np.ascontiguousarray(
            indices[b].reshape(tg, 128, K).transpose(1, 0, 2).reshape(128, tg * K)
        ).astype(np.int32)
        ghead = gate[hg * HPC:(hg + 1) * HPC]                           # [4]
        gpp = np.stack([np.repeat(ghead[0:2], D), np.repeat(ghead[2:4], D)],
                       axis=1).astype(np.float32)                       # [128,2]
        g1pp = np.tile((1.0 - ghead)[None, :], (128, 1)).astype(np.float32)
        in_maps.append(dict(xT=xT, wq=wq, wk=wk, wv=wv, wp=wp, qkb=qkb,
                            dbs=dbs, idx=idx, gpp=gpp, g1pp=g1pp))

    # host-side bias: bproj + ((1-gate) * bv) @ Wproj
    gexp = np.repeat(gate, D)                                           # [C]
    bv = bqkv[2 * C:]
    host_bias = bproj + ((1.0 - gexp) * bv) @ Wproj                     # [C]
    return in_maps, host_bias


def host_finalize(results, host_bias, t=T):
    out = np.zeros((B, t, C), np.float32)
    for b in range(B):
        acc = np.zeros((t, C), np.float64)
        for hg in range(4):
            acc += results[b * 4 + hg]["out"].astype(np.float64)
        out[b] = (acc + host_bias[None, :]).astype(np.float32)
    return out


_CACHED_NC = None


def kernel(**inputs) -> np.ndarray:
    global _CACHED_NC
    from concourse.bass_utils import run_bass_kernel_spmd
    if _CACHED_NC is None:
        _CACHED_NC = build_program()
    in_maps, host_bias = host_prepare(inputs)
    res = run_bass_kernel_spmd(_CACHED_NC, in_maps, list(range(N_CORES)))
    return host_finalize(res.results, host_bias)

